# revision 19
# baseline (speedup 1.0000x reference)
"""Trainium2 Bass kernel: classical single-head attention layer.

reference math:
    qkv = x @ w_qkv.T        # x [8192, 512], w_qkv [192, 512]
    q, k, v = split(qkv, 3)  # each [8192, 64]
    out = softmax(q @ k.T / 8) @ v   # [8192, 64]

Sharding: Q row-blocks across 8 cores (1024 rows each); K/V replicated.
Two NEFF passes (all matmul operands bf16):
  pass 1 (per core c): qkv = x-block @ W^T in the natural [seq, 192]
          layout, b-major so each seq-block's psum->sbuf cast overlaps
          the next block's matmuls.
  host:   split/transpose/concat Q/K/V across cores (bitwise bf16),
          build the folded K^T image and the V' image with ones column.
  pass 2 (per core c): flash-style attention for the core's 1024 queries.
          Per key-chunk PAIR (j, j+32): the two S^T matmuls run
          CONCURRENTLY via PE row tiling (K=64 contraction on array rows
          0:64 / 64:128) into one contiguous [128, 1024] psum tile
          (2 banks, each cleared by its own start=True), so the pair
          costs a single release-semaphore on its leader.  exp of both
          chunks runs as ONE instruction, alternating per slot between
          ACT (exact, scale folded into the affine) and DVE (Schraudolph
          bf16-bit trick, int16 round output), keeping both engines ~50%
          loaded and the PE the pacer.  P^T@V' with a ones column in V'
          accumulates (PV)^T + softmax denominator in PSUM; the
          transpose + reciprocal-scale tail runs in 2 dedicated psum
          banks so PE writes and DVE reads don't serialize.
"""

import math
import os
from contextlib import ExitStack

import ml_dtypes
import numpy as np

import concourse.bass as bass
import concourse.mybir as mybir
import concourse.tile as tile
from concourse import bacc
from concourse.bass_utils import run_bass_kernel_spmd
from concourse.masks import make_identity

F32 = mybir.dt.float32
BF16 = mybir.dt.bfloat16
I16 = mybir.dt.int16
BF = ml_dtypes.bfloat16

N = 8192          # sequence length
D_IN = 512        # input features
D = 64            # head dim (size_out)
NC = 8            # cores
SEQ_C = N // NC   # 1024 queries/keys per core
SCALE = 1.0 / math.sqrt(D)

# V' chunk stride in elements (65 used, padded so chunk starts are 32B-aligned)
VP_W = 80

# Schraudolph bf16 exp on DVE: bf16_bits(exp(x)) ~= x*SCH_C1 + SCH_C2,
# computed as one fused tensor_scalar with int16 (round) output
SCH_C1 = 128.0 / math.log(2.0)
SCH_C2 = 127.0 * 128.0 - 366393.0 / 65536.0

# pass-2 processing order: pair slots (s, s+32); vp is host-swizzled so the
# m-th processed chunk sits at column m*VP_W
CHUNK_ORDER = [j for s in range(32) for j in (s, s + 32)]

WARMUP1 = int(os.environ.get("ATTN_WARMUP1", "6"))
WARMUP2 = int(os.environ.get("ATTN_WARMUP2", "4"))

# stash of BassKernelResults for test harness introspection
LAST_RESULTS = []

_CACHE = {}


def _build_pass1():
    """Projection: xt [512, 1024] bf16, wt [512, 192] bf16 ->
    qkv image [128, 8*192] bf16 (row p, cols b*192.. hold qkv[b*128+p, :])."""
    nc = bacc.Bacc("TRN2", target_bir_lowering=False, debug=False, num_devices=NC)
    xt_d = [
        nc.dram_tensor(f"xt{i}", [128, SEQ_C], BF16, kind="ExternalInput")
        for i in range(4)
    ]
    wt_d = nc.dram_tensor("wt", [D_IN, 3 * D], BF16, kind="ExternalInput")
    qkv_d = nc.dram_tensor("qkv", [128, 8 * 3 * D], BF16, kind="ExternalOutput")

    with tile.TileContext(nc) as tc, ExitStack() as ctx:
        sb = ctx.enter_context(tc.tile_pool(name="sb", bufs=1))
        ps = ctx.enter_context(tc.tile_pool(name="ps", bufs=1, space="PSUM"))
        wsb = ctx.enter_context(tc.tile_pool(name="wsb", bufs=1))
        wps = ctx.enter_context(tc.tile_pool(name="wps", bufs=2, space="PSUM"))

        # warm-up matmuls emitted first (dependency-free)
        ident = wsb.tile([128, 128], F32)
        make_identity(nc, ident[:])
        for _ in range(WARMUP1):
            w = wps.tile([128, 128], F32, tag="warm")
            nc.tensor.matmul(w[:], ident[:], ident[:], start=True, stop=True)

        # w^T as [128, 4 * 192] (small, needed first)
        wt_sb = sb.tile([128, 4 * 3 * D], BF16)
        nc.sync.dma_start(
            wt_sb[:].rearrange("p (i o) -> p i o", i=4),
            wt_d.ap().rearrange("(i p) o -> p i o", p=128),
        )
        xt_sb = []
        for i in range(4):
            t = sb.tile([128, SEQ_C], BF16, tag=f"xt{i}")
            nc.sync.dma_start(t[:], xt_d[i][:, :])
            xt_sb.append(t)

        qkv_sb = sb.tile([128, 8 * 3 * D], BF16)

        # i-major: all blocks' chunk-i matmuls run as chunk i lands.
        # 2 blocks share a psum bank; only the first clears it (start=True
        # clears has_written for the WHOLE bank; clear bits -> fresh write).
        acc = []
        for t in range(4):
            a = ps.tile([128, 2 * 3 * D], F32, tag=f"acc{t}", name=f"acc{t}")
            acc.append(a)
        for i in range(4):
            for b in range(8):
                nc.tensor.matmul(
                    acc[b // 2][:, (b % 2) * 192 : (b % 2) * 192 + 192],
                    xt_sb[i][:, b * 128 : b * 128 + 128],
                    wt_sb[:, i * 192 : (i + 1) * 192],
                    start=(i == 0 and b % 2 == 0),
                    stop=(i == 3),
                    skip_group_check=True,
                )
        for b in range(8):
            nc.vector.tensor_copy(
                qkv_sb[:, b * 192 : (b + 1) * 192],
                acc[b // 2][:, (b % 2) * 192 : (b % 2) * 192 + 192],
            )
            if b % 4 == 3:
                nc.sync.dma_start(
                    qkv_d[:, (b - 3) * 192 : (b + 1) * 192],
                    qkv_sb[:, (b - 3) * 192 : (b + 1) * 192],
                )

    nc.compile()
    return nc


def _build_pass2():
    """Attention pass per core.

    inputs : qt2 [128, 1024] (Q^T duplicated on both partition halves)
             kt2 [128, 4096] (K^T: rows 0:64 keys 0:4096, rows 64:128 rest),
                 split into 4 column-chunk dram tensors for early start
             vp  [128, 64*VP_W] (V' image in CHUNK_ORDER, ones at col 64),
                 split into 4 chunk dram tensors
    output : out [1024, 64] f32
    """
    nc = bacc.Bacc("TRN2", target_bir_lowering=False, debug=False, num_devices=NC)
    qt_d = nc.dram_tensor("qt2", [128, SEQ_C], BF16, kind="ExternalInput")
    kt_d = [
        nc.dram_tensor(f"kt{i}", [128, 1024], BF16, kind="ExternalInput")
        for i in range(4)
    ]
    vp_d = [
        nc.dram_tensor(f"vp{i}", [128, 16 * VP_W], BF16, kind="ExternalInput")
        for i in range(4)
    ]
    ident_d = nc.dram_tensor("ident", [128, 128], F32, kind="ExternalInput")
    out_d = nc.dram_tensor("out", [SEQ_C, D], F32, kind="ExternalOutput")

    with tile.TileContext(nc) as tc, ExitStack() as ctx:
        sb = ctx.enter_context(tc.tile_pool(name="sb", bufs=1))
        p_pool = ctx.enter_context(tc.tile_pool(name="pT", bufs=4))
        o_sb_pool = ctx.enter_context(tc.tile_pool(name="osb", bufs=2))
        fin_pool = ctx.enter_context(tc.tile_pool(name="fin", bufs=4))
        s_pool = ctx.enter_context(tc.tile_pool(name="sT", bufs=3, space="PSUM"))
        o_pool = ctx.enter_context(tc.tile_pool(name="oac", bufs=2, space="PSUM"))

        # warm-up operand + exp-table scratch memsets first on DVE so the
        # PE warm-ups and the ACT table preload start immediately
        wdummy = sb.tile([128, 128], F32)
        nc.vector.memset(wdummy[:], 0.0)
        scratch = fin_pool.tile([1, 1], F32, tag="scr")
        nc.vector.memset(scratch[:], 0.0)
        warm = s_pool.tile([128, 1024], F32, tag="s2", name="warm")
        for _ in range(WARMUP2):
            nc.tensor.matmul(warm[:, :128], wdummy[:], wdummy[:], start=True, stop=True)
        # preload the exp table (ACT queue, ~2.7us, overlaps DMA)
        nc.scalar.activation(
            scratch[:], scratch[:], mybir.ActivationFunctionType.Exp
        )

        # input DMAs; issue split across the sync and (post-preload) ACT
        # queues so descriptor generation (~0.6us per dma_start) overlaps,
        # first-needed transfers first
        kta = sb.tile([128, 256], BF16)
        ktb1 = sb.tile([128, 256], BF16)
        ktb2 = sb.tile([128, 512], BF16)
        qta = sb.tile([128, 512], BF16)
        qtb = sb.tile([128, 512], BF16)
        vpa = sb.tile([128, 4 * VP_W], BF16)
        vpb = sb.tile([128, 12 * VP_W], BF16)
        kt_sb = [None] + [
            sb.tile([128, 1024], BF16, tag=f"kt{i}", name=f"kt{i}") for i in range(1, 4)
        ]
        vp_sb = [None] + [
            sb.tile([128, 16 * VP_W], BF16, tag=f"vp{i}", name=f"vp{i}")
            for i in range(1, 4)
        ]
        ident = sb.tile([128, 128], F32)
        nc.sync.dma_start(kta[:], kt_d[0][:, 0:256])
        nc.sync.dma_start(qta[:], qt_d[:, 0:512])
        nc.sync.dma_start(qtb[:], qt_d[:, 512:1024])
        nc.sync.dma_start(ktb1[:], kt_d[0][:, 256:512])
        nc.sync.dma_start(vpa[:], vp_d[0][:, : 4 * VP_W])
        nc.sync.dma_start(ktb2[:], kt_d[0][:, 512:1024])
        nc.sync.dma_start(vpb[:], vp_d[0][:, 4 * VP_W :])
        for i in range(1, 4):
            nc.sync.dma_start(kt_sb[i][:], kt_d[i][:, :])
            nc.scalar.dma_start(vp_sb[i][:], vp_d[i][:, :])
        nc.sync.dma_start(ident[:], ident_d[:, :])

        def kt_slice(half, sl):
            if sl < 2:
                t, col = kta, sl * 128
            elif sl < 4:
                t, col = ktb1, (sl - 2) * 128
            elif sl < 8:
                t, col = ktb2, (sl - 4) * 128
            else:
                t, col = kt_sb[sl // 8], (sl % 8) * 128
            return t[64 * half : 64 * half + 64, col : col + 128]

        def vp_slice(m):
            if m < 4:
                t, off = vpa, m * VP_W
            elif m < 16:
                t, off = vpb, (m - 4) * VP_W
            else:
                t, off = vp_sb[m // 16], (m % 16) * VP_W
            return t[:, off : off + D + 1]

        exp_f = mybir.ActivationFunctionType.Exp

        # interleave the two 512-query streams: per wall-slot sl emit
        #   S(q0,sl) S(q1,sl) | exp(q0,sl) exp(q1,sl) | PV(q0,sl-1) PV(q1,sl-1)
        # so each exp has a full wall-slot of PE work to hide behind, ACT and
        # DVE each take one exp per wall-slot, and S/PV runs batch across
        # streams (one leader semaphore per run).
        o_ps = [o_pool.tile([128, 512], F32, tag="o", name=f"o{q}") for q in (0, 1)]
        s2_t = {}
        p2_t = {}

        def emit_s(q, sl):
            s2 = s_pool.tile([128, 1024], F32, tag="s2", name="s2")
            s2_t[(q, sl)] = s2
            qt = qta if q == 0 else qtb
            nc.tensor.matmul(
                s2[:, 0:512], kt_slice(0, sl), qt[0:64, :],
                start=True, stop=True,
            )
            nc.tensor.matmul(
                s2[:, 512:1024], kt_slice(1, sl), qt[64:128, :],
                start=True, stop=True,
            )

        def emit_exp(q, sl):
            s2 = s2_t.pop((q, sl))
            p2 = p_pool.tile([128, 1024], BF16, tag="p2")
            p2_t[(q, sl)] = p2
            if (sl + q) % 2 == 0:
                nc.scalar.activation(p2[:], s2[:], exp_f, scale=SCALE)
            else:
                nc.vector.tensor_scalar(
                    p2[:].bitcast(I16),
                    s2[:],
                    SCH_C1 * SCALE,
                    SCH_C2,
                    op0=mybir.AluOpType.mult,
                    op1=mybir.AluOpType.add,
                )

        def emit_pv(q, sl):
            p2 = p2_t.pop((q, sl))
            nc.tensor.matmul(
                o_ps[q][0 : D + 1, :], vp_slice(2 * sl), p2[:, 0:512],
                start=(sl == 0), stop=False, skip_group_check=True,
            )
            nc.tensor.matmul(
                o_ps[q][0 : D + 1, :], vp_slice(2 * sl + 1), p2[:, 512:1024],
                start=False, stop=(sl == 31), skip_group_check=True,
            )

        for sl in range(32):
            emit_s(0, sl)
            emit_s(1, sl)
            emit_exp(0, sl)
            emit_exp(1, sl)
            if sl >= 1:
                emit_pv(0, sl - 1)
                emit_pv(1, sl - 1)
        emit_pv(0, 31)
        emit_pv(1, 31)

        for q in (0, 1):
            # o_ps rows 0:64 = (P V)^T, row 64 = softmax denominator
            o_sb = o_sb_pool.tile([D + 1, 512], F32)
            nc.scalar.copy(o_sb[:], o_ps[q][0 : D + 1, :])
            ot4 = fin_pool.tile([128, 4 * D], F32, tag="ot4")
            for t in range(4):
                # each transpose gets its own retired s2 ring tile so PE
                # writes and ACT/DVE reads never share a tile
                t_ps = s_pool.tile([128, 1024], F32, tag="s2", name="t_ps")
                tp = t_ps[:, 0 : D + 1]
                nc.tensor.transpose(
                    tp,
                    o_sb[:, t * 128 : (t + 1) * 128],
                    ident[: D + 1, : D + 1],
                )
                rec = fin_pool.tile([128, 1], F32, tag="rec")
                nc.vector.reciprocal(rec[:], tp[:, D : D + 1])
                nc.scalar.activation(
                    ot4[:, t * D : (t + 1) * D],
                    tp[:, :D],
                    mybir.ActivationFunctionType.Copy,
                    scale=rec[:],
                )
            r0 = q * 512
            nc.scalar.dma_start(
                out_d[r0 : r0 + 512, :].rearrange("(t p) o -> p t o", p=128),
                ot4[:].rearrange("p (t o) -> p t o", t=4),
            )

    nc.compile()
    return nc


def kernel(x: np.ndarray, w_qkv: np.ndarray) -> np.ndarray:
    global LAST_RESULTS
    LAST_RESULTS = []
    x = np.asarray(x, dtype=np.float32)
    w_qkv = np.asarray(w_qkv, dtype=np.float32)

    if "p1" not in _CACHE:
        _CACHE["p1"] = _build_pass1()
    if "p2" not in _CACHE:
        _CACHE["p2"] = _build_pass2()

    xt = np.ascontiguousarray(x.T).astype(BF)       # [512, 8192]
    wt = np.ascontiguousarray(w_qkv.T).astype(BF)   # [512, 192]

    in_maps1 = [
        {
            **{
                f"xt{i}": np.ascontiguousarray(
                    xt[i * 128 : (i + 1) * 128, c * SEQ_C : (c + 1) * SEQ_C]
                )
                for i in range(4)
            },
            "wt": wt,
        }
        for c in range(NC)
    ]
    res1 = run_bass_kernel_spmd(_CACHE["p1"], in_maps1, core_ids=list(range(NC)))
    LAST_RESULTS.append(res1)

    # qkv image [128, 8*192] -> [1024, 192] per core (bitwise bf16 ops only)
    qkv = [
        res1.results[c]["qkv"].reshape(128, 8, 3 * D).transpose(1, 0, 2).reshape(SEQ_C, 3 * D)
        for c in range(NC)
    ]
    k_full = np.concatenate([m[:, D : 2 * D] for m in qkv], axis=0)  # [8192, 64]
    v_full = np.concatenate([m[:, 2 * D : 3 * D] for m in qkv], axis=0)

    kt_full = np.ascontiguousarray(k_full.T)          # [64, 8192]
    # K^T folded to 128 partitions: rows 0:64 keys 0:4096, rows 64:128 the rest
    kt2 = np.ascontiguousarray(
        np.concatenate([kt_full[:, : N // 2], kt_full[:, N // 2 :]], axis=0)
    )
    # V' image [128, 64*VP_W]: m-th processed chunk j=CHUNK_ORDER[m] at cols
    # m*VP_W, ones at col 64 of each chunk
    vp = np.zeros((128, (N // 128) * VP_W), dtype=BF)
    for m, j in enumerate(CHUNK_ORDER):
        vp[:, m * VP_W : m * VP_W + D] = v_full[j * 128 : (j + 1) * 128, :]
        vp[:, m * VP_W + D] = 1.0

    ident_np = np.eye(128, dtype=np.float32)
    in_maps2 = []
    for c in range(NC):
        qt = np.ascontiguousarray(qkv[c][:, :D].T)    # [64, 1024]
        m = {
            "qt2": np.ascontiguousarray(np.concatenate([qt, qt], axis=0)),
            "ident": ident_np,
            **{
                f"kt{i}": np.ascontiguousarray(kt2[:, i * 1024 : (i + 1) * 1024])
                for i in range(4)
            },
            **{
                f"vp{i}": np.ascontiguousarray(
                    vp[:, i * 16 * VP_W : (i + 1) * 16 * VP_W]
                )
                for i in range(4)
            },
        }
        in_maps2.append(m)
    res2 = run_bass_kernel_spmd(_CACHE["p2"], in_maps2, core_ids=list(range(NC)))
    LAST_RESULTS.append(res2)

    out = np.concatenate([res2.results[c]["out"] for c in range(NC)], axis=0)
    return out.astype(np.float32)
